# revision 4
# baseline (speedup 1.0000x reference)
"""Causal multi-head attention (B=1, S=4096, D=2048, H=16) on 8 trn2 cores.

Sharding: tensor-parallel over heads (2 heads/core) for QKV + attention;
output projection is head-sharded (row-parallel wo) with a chunked
ReduceScatter over sequence rows; the host concatenates the 8 row-slices.

Numerics: fp32r matmuls (fp32 storage, reduced-precision PE path),
softmax in fp32 without max-subtraction (scores are O(+-6); masked
entries are simply never computed / zeroed), RoPE applied via
host-permuted weight columns so the rotation pairs sit in partition
halves (plain partition-block copies instead of stride-2 ops).
"""

import sys

for _p in ("/opt/trn_rl_repo", "/root/.axon_site/_ro/trn_rl_repo"):
    if _p not in sys.path:
        sys.path.insert(0, _p)

import numpy as np

import concourse.bacc as bacc
import concourse.mybir as mybir
from concourse.bass_utils import run_bass_kernel_spmd
from concourse.masks import make_identity
from concourse.tile import TileContext

F32 = mybir.dt.float32
F32R = mybir.dt.float32r

S = 4096
D = 2048
H = 16
HD = 128
NCORES = 8
HPC = H // NCORES  # heads per core = 2
CPC = HPC * HD  # cols per core = 256
ROPE_THETA = 10000.0
SCALE = 1.0 / float(np.sqrt(np.float32(HD)))

NSB = S // 512  # 8 q-blocks of 512
NDT = D // 128  # 16 contraction tiles
NKT = S // 128  # 32 key tiles
NST = S // 128  # 32 seq tiles


def _phase_a(nc, tc, tensors):
    """QKV projections + RoPE + V transpose."""
    xT = tensors["xT"]
    cosf = tensors["cosf"]
    sinf = tensors["sinf"]
    qT = tensors["qT"]
    kT = tensors["kT"]
    v_nat = tensors["v_nat"]
    ident = tensors["ident"]
    with (
        tc.tile_pool(name="wqkv", bufs=1) as wqkv,
        tc.tile_pool(name="xin", bufs=3) as xin,
        tc.tile_pool(name="trig", bufs=2) as trig,
        tc.tile_pool(name="rope", bufs=2) as rope,
        tc.tile_pool(name="vst", bufs=2) as vst,
        tc.tile_pool(name="psA", bufs=1, space="PSUM") as psA,
        tc.tile_pool(name="psT", bufs=2, space="PSUM") as psT,
    ):
        # weights -> SBUF f32r, layout [128, dt, 256]
        w_r = {}
        for wname in ("q", "k", "v"):
            w = tensors[f"w{wname}2"]
            wr = wqkv.tile([128, NDT * CPC], F32R, tag=f"w{wname}r", name=f"w{wname}r")
            w_r[wname] = wr
            wv_ = w.rearrange("(dt p) c -> p dt c", p=128)
            for g in range(4):  # 4 dt per chunk
                st = wqkv.tile([128, 4 * CPC], F32, tag="wqkvst", name="wqkvst")
                nc.sync.dma_start(out=st[:], in_=wv_[:, 4 * g : 4 * g + 4, :])
                nc.vector.tensor_copy(wr[:, 4 * g * CPC : 4 * (g + 1) * CPC], st[:])

        for sb in range(NSB):
            sl = slice(sb * 512, (sb + 1) * 512)
            cosS = trig.tile([128, 512], F32, tag="cosS", name="cosS")
            sinS = trig.tile([128, 512], F32, tag="sinS", name="sinS")
            nc.sync.dma_start(out=cosS[:], in_=cosf[:, sl])
            nc.sync.dma_start(out=sinS[:], in_=sinf[:, sl])

            ps = {
                t: psA.tile([128, 512], F32, tag=f"ps_{t}", name=f"ps_{t}")
                for t in ("q0", "q1", "k0", "k1", "v0", "v1")
            }
            for d in range(NDT):
                xf = xin.tile([128, 512], F32, tag="xf", name="xf")
                nc.sync.dma_start(out=xf[:], in_=xT[d * 128 : (d + 1) * 128, sl])
                xr = xin.tile([128, 512], F32R, tag="xr", name="xr")
                nc.vector.tensor_copy(xr[:], xf[:])
                st = dict(start=(d == 0), stop=(d == NDT - 1))
                for h in range(HPC):
                    co = d * CPC + h * HD
                    nc.tensor.matmul(
                        ps[f"q{h}"][:], w_r["q"][:, co : co + HD], xr[:], **st
                    )
                    nc.tensor.matmul(
                        ps[f"k{h}"][:], w_r["k"][:, co : co + HD], xr[:], **st
                    )
                    nc.tensor.matmul(
                        ps[f"v{h}"][:], w_r["v"][:, co : co + HD], xr[:], **st
                    )

            # RoPE for q/k: out = p*cosS + rot(p)*sinS  (rot swaps halves)
            for h in range(HPC):
                for t, dst in ((f"q{h}", qT[h]), (f"k{h}", kT[h])):
                    p = ps[t]
                    rot = rope.tile([128, 512], F32, tag="rot", name="rot")
                    nc.scalar.copy(rot[0:64, :], p[64:128, :])
                    nc.scalar.copy(rot[64:128, :], p[0:64, :])
                    t1 = rope.tile([128, 512], F32, tag="t1", name="t1")
                    nc.vector.tensor_mul(t1[:], p[:], cosS[:])
                    t2 = rope.tile([128, 512], F32, tag="t2", name="t2")
                    nc.vector.tensor_mul(t2[:], rot[:], sinS[:])
                    nc.vector.tensor_add(dst[:, sl], t1[:], t2[:])

            # v: copy psum -> staging, transpose 128x128 blocks -> v_nat
            for h in range(HPC):
                vs = vst.tile([128, 512], F32, tag="vs", name="vs")
                nc.scalar.copy(vs[:], ps[f"v{h}"][:])
                for j in range(4):
                    kt = 4 * sb + j
                    pt = psT.tile([128, 128], F32, tag="pt", name="pt")
                    nc.tensor.transpose(pt[:], vs[:, j * 128 : (j + 1) * 128], ident[:])
                    co = (h * NKT + kt) * 128
                    nc.scalar.copy(v_nat[:, co : co + 128], pt[:])


def _phase_b(nc, tc, tensors, aT):
    """Causal attention per head; writes normalized attn^T into aT."""
    qT = tensors["qT"]
    kT = tensors["kT"]
    v_nat = tensors["v_nat"]
    ones_r = tensors["ones_r"]
    with (
        tc.tile_pool(name="probs", bufs=3) as probs,
        tc.tile_pool(name="nrm", bufs=2) as nrm,
        tc.tile_pool(name="psB", bufs=2, space="PSUM") as psB,
    ):
        for h in range(HPC):
            for qb in range(NSB):
                qsl = slice(qb * 512, (qb + 1) * 512)
                nkt = 4 * qb + 4
                po = psB.tile([128, 512], F32, tag="po", name="po")
                psum = psB.tile([1, 512], F32, tag="psum", name="psum")
                for kt in range(nkt):
                    ss = psB.tile([128, 512], F32, tag="ss", name="ss")
                    nc.tensor.matmul(
                        ss[:],
                        kT[h][:, kt * 128 : (kt + 1) * 128],
                        qT[h][:, qsl],
                        start=True,
                        stop=True,
                    )
                    pr = probs.tile([128, 512], F32R, tag="pr", name="pr")
                    nc.scalar.activation(
                        pr[:], ss[:], mybir.ActivationFunctionType.Exp, scale=SCALE
                    )
                    if kt >= 4 * qb:
                        # keep element iff q >= k:
                        # (qb*512 - kt*128) + f - p >= 0
                        nc.gpsimd.affine_select(
                            out=pr[:],
                            in_=pr[:],
                            compare_op=mybir.AluOpType.is_ge,
                            fill=0.0,
                            base=qb * 512 - kt * 128,
                            pattern=[[1, 512]],
                            channel_multiplier=-1,
                        )
                    acc = dict(start=(kt == 0), stop=(kt == nkt - 1))
                    nc.tensor.matmul(psum[:], ones_r[:], pr[:], **acc)
                    co = (h * NKT + kt) * 128
                    nc.tensor.matmul(po[:], v_nat[:, co : co + 128], pr[:], **acc)
                s_sb = nrm.tile([1, 512], F32, tag="s_sb", name="s_sb")
                nc.scalar.copy(s_sb[:], psum[:])
                bcn = nrm.tile([128, 512], F32, tag="bcn", name="bcn")
                nc.gpsimd.partition_broadcast(bcn[:], s_sb[:])
                rcn = nrm.tile([128, 512], F32, tag="rcn", name="rcn")
                nc.vector.reciprocal(rcn[:], bcn[:])
                nc.vector.tensor_mul(aT[h][:, qsl], po[:], rcn[:])


def _phase_cd(nc, tc, tensors, aT, dram_parts, rs_outs, out_part, groups):
    """Head-sharded out-projection + chunked ReduceScatter over seq rows."""
    wo_r = tensors["wo_r"]
    with (
        tc.tile_pool(name="osb", bufs=4) as osb,
        tc.tile_pool(name="psC", bufs=4, space="PSUM") as psC,
    ):
        for db in range(4):
            dsl = slice(db * 512, (db + 1) * 512)
            for stt in range(NST):
                pc = psC.tile([128, 512], F32, tag="pc", name="pc")
                for h in range(HPC):
                    nc.tensor.matmul(
                        pc[:],
                        aT[h][:, stt * 128 : (stt + 1) * 128],
                        wo_r[:, h * D + db * 512 : h * D + (db + 1) * 512],
                        start=(h == 0),
                        stop=(h == HPC - 1),
                    )
                oc = osb.tile([128, 512], F32, tag="oc", name="oc")
                if stt % 2 == 0:
                    nc.scalar.copy(oc[:], pc[:])
                else:
                    nc.vector.tensor_copy(oc[:], pc[:])
                nc.sync.dma_start(
                    out=dram_parts[db][stt * 128 : (stt + 1) * 128, :], in_=oc[:]
                )
            nc.gpsimd.collective_compute(
                "ReduceScatter",
                mybir.AluOpType.add,
                replica_groups=groups,
                ins=[dram_parts[db].opt()],
                outs=[rs_outs[db].opt()],
            )
            nc.sync.dma_start(out=out_part[:, dsl], in_=rs_outs[db][:])


def _build_program():
    nc = bacc.Bacc("TRN2", target_bir_lowering=False)

    tensors = {}
    tensors["xT"] = nc.dram_tensor("xT", [D, S], F32, kind="ExternalInput")
    tensors["wq2"] = nc.dram_tensor("wq2", [D, CPC], F32, kind="ExternalInput")
    tensors["wk2"] = nc.dram_tensor("wk2", [D, CPC], F32, kind="ExternalInput")
    tensors["wv2"] = nc.dram_tensor("wv2", [D, CPC], F32, kind="ExternalInput")
    tensors["wo2"] = nc.dram_tensor("wo2", [CPC, D], F32, kind="ExternalInput")
    tensors["cosf"] = nc.dram_tensor("cosf", [HD, S], F32, kind="ExternalInput")
    tensors["sinf"] = nc.dram_tensor("sinf", [HD, S], F32, kind="ExternalInput")
    out_part = nc.dram_tensor("out_part", [S // NCORES, D], F32, kind="ExternalOutput")

    groups = [list(range(NCORES))]

    with TileContext(nc) as tc:
        with (
            tc.tile_pool(name="persist", bufs=1) as persist,
            tc.tile_pool(name="dram", bufs=1, space="DRAM") as dram,
        ):
            tensors["qT"] = [
                persist.tile([128, S], F32R, tag=f"qT{h}", name=f"qT{h}")
                for h in range(HPC)
            ]
            tensors["kT"] = [
                persist.tile([128, S], F32R, tag=f"kT{h}", name=f"kT{h}")
                for h in range(HPC)
            ]
            # V natural [k-pos, hd] per (head, ktile): col block (h*NKT+kt)*128
            tensors["v_nat"] = persist.tile([128, HPC * S], F32R, tag="v_nat", name="v_nat")
            tensors["wo_r"] = persist.tile([128, HPC * D], F32R, tag="wo_r", name="wo_r")
            tensors["ones_r"] = persist.tile([128, 1], F32R, tag="ones_r", name="ones_r")
            tensors["ident"] = persist.tile([128, 128], F32, tag="ident", name="ident")
            make_identity(nc, tensors["ident"][:])

            ones_f = persist.tile([128, 1], F32, tag="ones_f", name="ones_f")
            nc.any.memset(ones_f[:], 1.0)
            nc.vector.tensor_copy(tensors["ones_r"][:], ones_f[:])

            # wo: [256, 2048] -> [128, 2, 2048] f32r
            with tc.tile_pool(name="wstage", bufs=2) as wstage:
                wo_v = tensors["wo2"].rearrange("(g p) d -> p g d", p=128)
                for g in range(HPC):
                    st = wstage.tile([128, D], F32, tag="wst", name="wst")
                    nc.sync.dma_start(out=st[:], in_=wo_v[:, g, :])
                    nc.vector.tensor_copy(tensors["wo_r"][:, g * D : (g + 1) * D], st[:])

            dram_parts = [
                dram.tile([S, 512], F32, tag=f"part{db}", name=f"part{db}")
                for db in range(4)
            ]
            rs_outs = [
                dram.tile([S // NCORES, 512], F32, tag=f"rso{db}", name=f"rso{db}")
                for db in range(4)
            ]

            _phase_a(nc, tc, tensors)

            with tc.tile_pool(name="attn", bufs=1) as attn:
                aT = [
                    attn.tile([128, S], F32R, tag=f"aT{h}", name=f"aT{h}")
                    for h in range(HPC)
                ]
                _phase_b(nc, tc, tensors, aT)
                _phase_cd(nc, tc, tensors, aT, dram_parts, rs_outs, out_part, groups)

    nc.compile()
    return nc


_NC_CACHE = None


def _get_program():
    global _NC_CACHE
    if _NC_CACHE is None:
        _NC_CACHE = _build_program()
    return _NC_CACHE


def _rope_tables():
    # match reference's f32 arithmetic
    i = np.arange(0, HD, 2, dtype=np.float32) / np.float32(HD)
    freqs = (np.float32(1.0) / np.float32(ROPE_THETA) ** i).astype(np.float32)  # [64]
    ang = np.arange(S, dtype=np.float32)[:, None] * freqs[None, :]  # [S, 64]
    cos = np.cos(ang).astype(np.float32).T  # [64, S]
    sin = np.sin(ang).astype(np.float32).T
    cosf = np.concatenate([cos, cos], axis=0)  # [128, S]
    sinf = np.concatenate([-sin, sin], axis=0)
    return np.ascontiguousarray(cosf), np.ascontiguousarray(sinf)


def kernel(x, mask, wq, wk, wv, wo):
    # mask is the standard causal mask produced by setup_inputs; causality is
    # implemented directly in the device program, so the tensor itself is not
    # shipped to the cores.
    x = np.asarray(x, dtype=np.float32)
    wq = np.asarray(wq, dtype=np.float32)
    wk = np.asarray(wk, dtype=np.float32)
    wv = np.asarray(wv, dtype=np.float32)
    wo = np.asarray(wo, dtype=np.float32)

    xT = np.ascontiguousarray(x.reshape(S, D).T)  # [D, S]

    # de-interleave permutation within each head (RoPE pairs -> halves)
    idx = np.concatenate([np.arange(0, HD, 2), np.arange(1, HD, 2)])
    perm = np.concatenate([h * HD + idx for h in range(H)])
    wq_p = wq[:, perm]
    wk_p = wk[:, perm]

    cosf, sinf = _rope_tables()

    nc = _get_program()
    in_maps = []
    for c in range(NCORES):
        csl = slice(c * CPC, (c + 1) * CPC)
        in_maps.append(
            {
                "xT": xT,
                "wq2": np.ascontiguousarray(wq_p[:, csl]),
                "wk2": np.ascontiguousarray(wk_p[:, csl]),
                "wv2": np.ascontiguousarray(wv[:, csl]),
                "wo2": np.ascontiguousarray(wo[csl, :]),
                "cosf": cosf,
                "sinf": sinf,
            }
        )
    res = run_bass_kernel_spmd(nc, in_maps, core_ids=list(range(NCORES)))
    out = np.concatenate([res.results[c]["out_part"] for c in range(NCORES)], axis=0)
    return out.reshape(1, S, D).astype(np.float32)


# revision 11
# speedup vs baseline: 1.3620x; 1.3620x over previous
"""Causal multi-head attention (B=1, S=4096, D=2048, H=16) on 8 trn2 cores.

Sharding: tensor-parallel over heads (2 heads/core) for QKV + attention;
output projection is head-sharded (row-parallel wo) with a chunked
ReduceScatter over sequence rows; the host concatenates the 8 row-slices.

Numerics: fp32r matmuls (fp32 storage, reduced-precision PE path),
softmax in fp32 without max-subtraction (scores are O(+-6); masked
entries are simply never computed / zeroed), RoPE applied via
host-permuted weight columns so the rotation pairs sit in partition
halves (plain partition-block copies instead of stride-2 ops).
"""

import sys

for _p in ("/opt/trn_rl_repo", "/root/.axon_site/_ro/trn_rl_repo"):
    if _p not in sys.path:
        sys.path.insert(0, _p)

import numpy as np

import concourse.bacc as bacc
import concourse.mybir as mybir
from concourse.bass_utils import run_bass_kernel_spmd
from concourse.masks import make_identity
from concourse.tile import TileContext

F32 = mybir.dt.float32
F32R = mybir.dt.float32r

S = 4096
D = 2048
H = 16
HD = 128
NCORES = 8
HPC = H // NCORES  # heads per core = 2
CPC = HPC * HD  # cols per core = 256
ROPE_THETA = 10000.0
SCALE = 1.0 / float(np.sqrt(np.float32(HD)))

NSB = S // 512  # 8 q-blocks of 512
NDT = D // 128  # 16 contraction tiles
NKT = S // 128  # 32 key tiles
NST = S // 128  # 32 seq tiles


def _phase_a(nc, tc, tensors):
    """QKV projections + RoPE + V transpose."""
    xT = tensors["xT"]
    cosf = tensors["cosf"]
    sinf = tensors["sinf"]
    qT = tensors["qT"]
    kT = tensors["kT"]
    v_nat = tensors["v_nat"]
    ident = tensors["ident"]
    with (
        tc.tile_pool(name="wqkv", bufs=1) as wqkv,
        tc.tile_pool(name="xin", bufs=3) as xin,
        tc.tile_pool(name="trig", bufs=2) as trig,
        tc.tile_pool(name="rope", bufs=2) as rope,
        tc.tile_pool(name="vst", bufs=2) as vst,
        tc.tile_pool(name="psA", bufs=1, space="PSUM") as psA,
        tc.tile_pool(name="psT", bufs=2, space="PSUM") as psT,
    ):
        # weights -> SBUF f32r, layout [128, dt, 256]
        w_r = {}
        for wname in ("q", "k", "v"):
            w = tensors[f"w{wname}2"]
            wr = wqkv.tile([128, NDT * CPC], F32R, tag=f"w{wname}r", name=f"w{wname}r")
            w_r[wname] = wr
            wv_ = w.rearrange("(dt p) c -> p dt c", p=128)
            for g in range(4):  # 4 dt per chunk
                st = wqkv.tile([128, 4 * CPC], F32, tag="wqkvst", name="wqkvst")
                nc.sync.dma_start(out=st[:], in_=wv_[:, 4 * g : 4 * g + 4, :])
                nc.vector.tensor_copy(wr[:, 4 * g * CPC : 4 * (g + 1) * CPC], st[:])

        for sb in range(NSB):
            sl = slice(sb * 512, (sb + 1) * 512)
            cosS = trig.tile([128, 512], F32, tag="cosS", name="cosS")
            sinS = trig.tile([128, 512], F32, tag="sinS", name="sinS")
            nc.sync.dma_start(out=cosS[:], in_=cosf[:, sl])
            nc.sync.dma_start(out=sinS[:], in_=sinf[:, sl])

            ps = {
                t: psA.tile([128, 512], F32, tag=f"ps_{t}", name=f"ps_{t}")
                for t in ("q0", "q1", "k0", "k1", "v0", "v1")
            }
            for d in range(NDT):
                xf = xin.tile([128, 512], F32, tag="xf", name="xf")
                nc.sync.dma_start(out=xf[:], in_=xT[d * 128 : (d + 1) * 128, sl])
                xr = xin.tile([128, 512], F32R, tag="xr", name="xr")
                nc.vector.tensor_copy(xr[:], xf[:])
                st = dict(start=(d == 0), stop=(d == NDT - 1))
                for h in range(HPC):
                    co = d * CPC + h * HD
                    nc.tensor.matmul(
                        ps[f"q{h}"][:], w_r["q"][:, co : co + HD], xr[:], **st
                    )
                    nc.tensor.matmul(
                        ps[f"k{h}"][:], w_r["k"][:, co : co + HD], xr[:], **st
                    )
                    nc.tensor.matmul(
                        ps[f"v{h}"][:], w_r["v"][:, co : co + HD], xr[:], **st
                    )

            # RoPE for q/k: out = p*cosS + rot(p)*sinS  (rot swaps halves)
            for h in range(HPC):
                for t, dst in ((f"q{h}", qT[h]), (f"k{h}", kT[h])):
                    p = ps[t]
                    rot = rope.tile([128, 512], F32, tag="rot", name="rot")
                    nc.scalar.copy(rot[0:64, :], p[64:128, :])
                    nc.scalar.copy(rot[64:128, :], p[0:64, :])
                    t1 = rope.tile([128, 512], F32, tag="t1", name="t1")
                    nc.vector.tensor_mul(t1[:], p[:], cosS[:])
                    t2 = rope.tile([128, 512], F32, tag="t2", name="t2")
                    nc.vector.tensor_mul(t2[:], rot[:], sinS[:])
                    nc.vector.tensor_add(dst[:, sl], t1[:], t2[:])

            # v: copy psum -> staging, transpose 128x128 blocks -> v_nat
            for h in range(HPC):
                vs = vst.tile([128, 512], F32, tag="vs", name="vs")
                nc.scalar.copy(vs[:], ps[f"v{h}"][:])
                for j in range(4):
                    kt = 4 * sb + j
                    pt = psT.tile([128, 128], F32, tag="pt", name="pt")
                    nc.tensor.transpose(pt[:], vs[:, j * 128 : (j + 1) * 128], ident[:])
                    co = (h * NKT + kt) * 128
                    nc.scalar.copy(v_nat[:, co : co + 128], pt[:])


def _phase_b(nc, tc, tensors, aT, a2a_ins, a2a_outs, groups):
    """Causal attention per head; writes normalized attn^T into aT and
    kicks off the per-head AllToAll as soon as that head finishes."""
    qT = tensors["qT"]
    kT = tensors["kT"]
    v_nat = tensors["v_nat"]
    ones_r = tensors["ones_r"]
    with (
        tc.tile_pool(name="probs", bufs=3) as probs,
        tc.tile_pool(name="nrm", bufs=2) as nrm,
        tc.tile_pool(name="psB", bufs=2, space="PSUM") as psB,
    ):
        for h in range(HPC):
            for qb in range(NSB):
                qsl = slice(qb * 512, (qb + 1) * 512)
                nkt = 4 * qb + 4
                po = psB.tile([128, 512], F32, tag="po", name="po")
                psum = psB.tile([1, 512], F32, tag="psum", name="psum")
                for kt in range(nkt):
                    ss = psB.tile([128, 512], F32, tag="ss", name="ss")
                    nc.tensor.matmul(
                        ss[:],
                        kT[h][:, kt * 128 : (kt + 1) * 128],
                        qT[h][:, qsl],
                        start=True,
                        stop=True,
                    )
                    pr = probs.tile([128, 512], F32R, tag="pr", name="pr")
                    nc.scalar.activation(
                        pr[:], ss[:], mybir.ActivationFunctionType.Exp, scale=SCALE
                    )
                    if kt >= 4 * qb:
                        # keep element iff q >= k:
                        # (qb*512 - kt*128) + f - p >= 0
                        nc.gpsimd.affine_select(
                            out=pr[:],
                            in_=pr[:],
                            compare_op=mybir.AluOpType.is_ge,
                            fill=0.0,
                            base=qb * 512 - kt * 128,
                            pattern=[[1, 512]],
                            channel_multiplier=-1,
                        )
                    acc = dict(start=(kt == 0), stop=(kt == nkt - 1))
                    nc.tensor.matmul(psum[:], ones_r[:], pr[:], **acc)
                    co = (h * NKT + kt) * 128
                    nc.tensor.matmul(po[:], v_nat[:, co : co + 128], pr[:], **acc)
                s_sb = nrm.tile([1, 512], F32, tag="s_sb", name="s_sb")
                nc.scalar.copy(s_sb[:], psum[:])
                rcs = nrm.tile([1, 512], F32, tag="rcs", name="rcs")
                nc.vector.reciprocal_approx_fast(out=rcs[:], in_=s_sb[:])
                bcn = nrm.tile([128, 512], F32, tag="bcn", name="bcn")
                nc.gpsimd.partition_broadcast(bcn[:], rcs[:])
                nc.vector.tensor_mul(aT[h][:, qsl], po[:], bcn[:])

            # ship this head's attn^T to the owning cores: block j of the
            # AllToAll input = aT[h][:, q-slice j] -> core j
            nc.sync.dma_start(
                out=a2a_ins[h].rearrange("(j p) q -> p j q", p=128),
                in_=aT[h][:].rearrange("p (j q) -> p j q", j=NCORES),
            )
            nc.gpsimd.collective_compute(
                "AllToAll",
                mybir.AluOpType.bypass,
                replica_groups=groups,
                ins=[a2a_ins[h].opt()],
                outs=[a2a_outs[h].opt()],
            )


def _phase_c(nc, tc, tensors, a2a_outs, out_part):
    """Out-projection for this core's own 512 output rows.

    a2a_outs[h] rows c*128..c*128+127 hold core c's attn^T (head h) for
    our q-slice; contraction pairs them with wo rows c*256 + h*128.
    """
    wo_full = tensors["wo_full"]
    with (
        tc.tile_pool(name="a2asb", bufs=1) as a2asb,
        tc.tile_pool(name="wosb", bufs=3) as wosb,
        tc.tile_pool(name="osb", bufs=4) as osb,
        tc.tile_pool(name="psC", bufs=2, space="PSUM") as psC,
    ):
        a2a_r = []
        for h in range(HPC):
            # payload was rounded to f32r before the AllToAll; plain copy
            ar = a2asb.tile([128, NCORES * 512], F32R, tag=f"a2r{h}", name=f"a2r{h}")
            nc.sync.dma_start(
                out=ar[:].rearrange("p (c q) -> p c q", c=NCORES),
                in_=a2a_outs[h].rearrange("(c p) q -> p c q", p=128),
            )
            a2a_r.append(ar)

        for db in range(4):
            pcs = [
                psC.tile([128, 512], F32, tag=f"pc{ss}", name=f"pc{ss}")
                for ss in range(4)
            ]
            first = True
            for c in range(NCORES):
                for h in range(HPC):
                    wt = wosb.tile([128, 512], F32, tag="wt", name="wt")
                    ro = c * CPC + h * HD
                    nc.sync.dma_start(
                        out=wt[:],
                        in_=wo_full[ro : ro + 128, db * 512 : (db + 1) * 512],
                    )
                    wr = wosb.tile([128, 512], F32R, tag="wr", name="wr")
                    nc.vector.tensor_copy(wr[:], wt[:])
                    last = c == NCORES - 1 and h == HPC - 1
                    for ss in range(4):
                        nc.tensor.matmul(
                            pcs[ss][:],
                            a2a_r[h][:, c * 512 + ss * 128 : c * 512 + (ss + 1) * 128],
                            wr[:],
                            start=first,
                            stop=last,
                        )
                    first = False
            for ss in range(4):
                oc = osb.tile([128, 512], F32, tag="oc", name="oc")
                if ss % 2 == 0:
                    nc.scalar.copy(oc[:], pcs[ss][:])
                else:
                    nc.vector.tensor_copy(oc[:], pcs[ss][:])
                nc.sync.dma_start(
                    out=out_part[
                        ss * 128 : (ss + 1) * 128, db * 512 : (db + 1) * 512
                    ],
                    in_=oc[:],
                )


def _build_program():
    nc = bacc.Bacc("TRN2", target_bir_lowering=False)

    tensors = {}
    tensors["xT"] = nc.dram_tensor("xT", [D, S], F32, kind="ExternalInput")
    tensors["wq2"] = nc.dram_tensor("wq2", [D, CPC], F32, kind="ExternalInput")
    tensors["wk2"] = nc.dram_tensor("wk2", [D, CPC], F32, kind="ExternalInput")
    tensors["wv2"] = nc.dram_tensor("wv2", [D, CPC], F32, kind="ExternalInput")
    tensors["wo_full"] = nc.dram_tensor("wo_full", [D, D], F32, kind="ExternalInput")
    tensors["cosf"] = nc.dram_tensor("cosf", [HD, S], F32, kind="ExternalInput")
    tensors["sinf"] = nc.dram_tensor("sinf", [HD, S], F32, kind="ExternalInput")
    out_part = nc.dram_tensor("out_part", [S // NCORES, D], F32, kind="ExternalOutput")

    groups = [list(range(NCORES))]

    with TileContext(nc) as tc:
        with (
            tc.tile_pool(name="persist", bufs=1) as persist,
            tc.tile_pool(name="dram", bufs=1, space="DRAM") as dram,
        ):
            tensors["qT"] = [
                persist.tile([128, S], F32R, tag=f"qT{h}", name=f"qT{h}")
                for h in range(HPC)
            ]
            tensors["kT"] = [
                persist.tile([128, S], F32R, tag=f"kT{h}", name=f"kT{h}")
                for h in range(HPC)
            ]
            # V natural [k-pos, hd] per (head, ktile): col block (h*NKT+kt)*128
            tensors["v_nat"] = persist.tile([128, HPC * S], F32R, tag="v_nat", name="v_nat")
            tensors["ones_r"] = persist.tile([128, 1], F32R, tag="ones_r", name="ones_r")
            tensors["ident"] = persist.tile([128, 128], F32, tag="ident", name="ident")
            make_identity(nc, tensors["ident"][:])

            ones_f = persist.tile([128, 1], F32, tag="ones_f", name="ones_f")
            nc.any.memset(ones_f[:], 1.0)
            nc.vector.tensor_copy(tensors["ones_r"][:], ones_f[:])

            a2a_ins = [
                dram.tile([NCORES * 128, 512], F32R, tag=f"a2i{h}", name=f"a2i{h}")
                for h in range(HPC)
            ]
            a2a_outs = [
                dram.tile([NCORES * 128, 512], F32R, tag=f"a2o{h}", name=f"a2o{h}")
                for h in range(HPC)
            ]

            _phase_a(nc, tc, tensors)

            with tc.tile_pool(name="attn", bufs=1) as attn:
                aT = [
                    attn.tile([128, S], F32R, tag=f"aT{h}", name=f"aT{h}")
                    for h in range(HPC)
                ]
                _phase_b(nc, tc, tensors, aT, a2a_ins, a2a_outs, groups)
            _phase_c(nc, tc, tensors, a2a_outs, out_part)

    nc.compile()
    return nc


_NC_CACHE = None


def _get_program():
    global _NC_CACHE
    if _NC_CACHE is None:
        _NC_CACHE = _build_program()
    return _NC_CACHE


def _rope_tables():
    # match reference's f32 arithmetic
    i = np.arange(0, HD, 2, dtype=np.float32) / np.float32(HD)
    freqs = (np.float32(1.0) / np.float32(ROPE_THETA) ** i).astype(np.float32)  # [64]
    ang = np.arange(S, dtype=np.float32)[:, None] * freqs[None, :]  # [S, 64]
    cos = np.cos(ang).astype(np.float32).T  # [64, S]
    sin = np.sin(ang).astype(np.float32).T
    cosf = np.concatenate([cos, cos], axis=0)  # [128, S]
    sinf = np.concatenate([-sin, sin], axis=0)
    return np.ascontiguousarray(cosf), np.ascontiguousarray(sinf)


def kernel(x, mask, wq, wk, wv, wo):
    # mask is the standard causal mask produced by setup_inputs; causality is
    # implemented directly in the device program, so the tensor itself is not
    # shipped to the cores.
    x = np.asarray(x, dtype=np.float32)
    wq = np.asarray(wq, dtype=np.float32)
    wk = np.asarray(wk, dtype=np.float32)
    wv = np.asarray(wv, dtype=np.float32)
    wo = np.asarray(wo, dtype=np.float32)

    xT = np.ascontiguousarray(x.reshape(S, D).T)  # [D, S]

    # de-interleave permutation within each head (RoPE pairs -> halves)
    idx = np.concatenate([np.arange(0, HD, 2), np.arange(1, HD, 2)])
    perm = np.concatenate([h * HD + idx for h in range(H)])
    wq_p = wq[:, perm]
    wk_p = wk[:, perm]

    cosf, sinf = _rope_tables()

    nc = _get_program()
    in_maps = []
    for c in range(NCORES):
        csl = slice(c * CPC, (c + 1) * CPC)
        in_maps.append(
            {
                "xT": xT,
                "wq2": np.ascontiguousarray(wq_p[:, csl]),
                "wk2": np.ascontiguousarray(wk_p[:, csl]),
                "wv2": np.ascontiguousarray(wv[:, csl]),
                "wo_full": wo,
                "cosf": cosf,
                "sinf": sinf,
            }
        )
    res = run_bass_kernel_spmd(nc, in_maps, core_ids=list(range(NCORES)))
    out = np.concatenate([res.results[c]["out_part"] for c in range(NCORES)], axis=0)
    return out.reshape(1, S, D).astype(np.float32)
